# revision 1
# baseline (speedup 1.0000x reference)
"""Multi-scale patch pooling (gather + segment-mean) for CLIP-AD on 8 trn2 cores.

Reference, per batch element b:
    large[b, g, :] = mean over l of tokens[b, large_mask[l, g], :]   (9-elt mean, 169 groups)
    mid[b, g, :]   = mean over l of tokens[b, mid_mask[l, g], :]     (4-elt mean, 196 groups)
    cls[b, 0, :]   = mean over t of tokens[b, t, :]                  (225-elt mean)
    out = concat([large, mid, cls], axis=1)                          # [B, 366, D]

Per batch this is out_b = W @ tokens_b where W is a tiny [366, 225] membership
matrix built host-side from the masks (count/L entries — handles arbitrary /
duplicate indices; the 1/L mean scale is folded in). The whole pipeline runs in
bf16 (inputs cast host-side, output cast back on host): quantization costs
~2.6e-3 relative error against a 2e-2 budget, and halves HBM traffic — this
problem is memory-bound (68 MB/core at ~360 GB/s ≈ 190 us floor).

Device kernel (per core, 64 batches = 16 quads of 4):
  * All bulk DMA on gpsimd/SWDGE (HWDGE's PDMA2D path pins a whole transfer to
    ONE engine at ~27 GB/s — measured, avoid). SWDGE assigns an HBM->SBUF
    descriptor to the SDMA engine owning its SBUF offset chunk (14336 B
    granularity), so ONE load = ONE engine: aggregate load bandwidth comes
    from many loads in flight on address-distinct pools (6 slots here), and
    the first two quads are split into 4 column-quarter DMAs into different
    slots (4 engines each) to cut the first-data latency ~4x.
  * SBUF->HBM descriptors are keyed by DRAM address (~655 KB chunks): one
    2.6 MB store per quad spans ~4 chunks, and the out tensor's per-quad
    stride is padded to ~4.5 chunks so consecutive stores rotate engines.
  * Stores are emitted one quad late so their sem-waits are satisfied on
    arrival — the gpsimd queue is in-order and a waiting store would block
    every load queued behind it. Load lookahead ramps up gradually: a large
    initial descriptor burst starves the engines' instruction-fetch DMAs.
  * Matmul on PE in bf16 (full-rate): for each m-tile mi (groups 3p+mi via a
    host-side column permute of W) and batch, accumulate the two k-chunks
    (113+112 token rows) into a [122, 896] PSUM tile (2 banks; each matmul N
    tile 512/384 sits within one bank). 48 back-to-back matmuls per quad.
  * PSUM evacuation alternates DVE / ACT, casting f32 -> bf16 into a [122,
    4*3*896] o tile per quad. Host untangles the layout, casts back to f32.

Sharding: pure data parallel on batch — 64 batches per core; W replicated.
"""

import numpy as np

B, T, D = 512, 225, 896
GL, LL = 169, 9
GM, LM = 196, 4
G = GL + GM + 1  # 366
N_CORES = 8
BP = B // N_CORES  # 64
QB = 4             # batches per quad
NQ = BP // QB      # 16 quads per core

KP = 113                      # k-chunk partition count (225 -> 113 + 112)
MP = G // 3                   # 122 partitions per m-tile (groups strided by 3)
_K_TILES = ((0, 113), (113, 112))
_N_TILES = ((0, 512), (512, 384))
ROWE = 2 * QB * D             # packed row elems per partition (7168)
ROWPAD = 32                   # bf16 elems of pad per packed row (64 B)
QCOL = ROWE // 4              # column-quarter elems (1792)
OCOL = QB * 3 * D             # o-tile cols per quad (10752)
OPAD = 15                     # pad rows per out quad: stride ~4.5 DRAM engine
                              # chunks so consecutive stores rotate engines
NSPLIT = 2                    # leading quads loaded as 4 column-quarters

_CACHE = {}


def _get_nc():
    if "nc" in _CACHE:
        return _CACHE["nc"]
    from contextlib import ExitStack

    import concourse.bacc as bacc
    import concourse.mybir as mybir
    import concourse.tile as tile

    f32 = mybir.dt.float32
    bf16 = mybir.dt.bfloat16

    nc = bacc.Bacc("TRN2", target_bir_lowering=False, debug=False)
    # tokq[q, p, ki*4D + b*D + d] = bf16 token (4q+b, ki*113+p, d)
    tokq = nc.dram_tensor(
        "tokq", [NQ, KP, ROWE + ROWPAD], bf16, kind="ExternalInput"
    ).ap()
    # w01T[t, mi*122 + p] = (count/L) of group 3p+mi at token t
    w01T = nc.dram_tensor("w01T", [T, G], bf16, kind="ExternalInput").ap()
    out = nc.dram_tensor(
        "out", [NQ, MP + OPAD, OCOL], bf16, kind="ExternalOutput"
    ).ap()

    NTOK = 6  # steady-state token quad slots (one chunk/engine each)
    NOB = 4   # o-tile slots

    with tile.TileContext(nc) as tc:
        with ExitStack() as ctx:
            # Allocation order fixes SBUF offsets: quarter pools first (8
            # distinct chunks for the startup quads), then the steady-state
            # slots, then o tiles / W at the top.
            qt_pools = [
                ctx.enter_context(tc.tile_pool(name=f"qt{j}", bufs=1))
                for j in range(4 * NSPLIT)
            ]
            tok_pools = [
                ctx.enter_context(tc.tile_pool(name=f"tokp{s}", bufs=1))
                for s in range(NTOK)
            ]
            obp = ctx.enter_context(tc.tile_pool(name="ob", bufs=NOB))
            wp = ctx.enter_context(tc.tile_pool(name="w", bufs=1))
            psp = ctx.enter_context(tc.tile_pool(name="ps", bufs=4, space="PSUM"))

            # Warm-up ops: first ACT/DVE instructions pick up table-load waits
            # in lowering; give them dummies with no cross-engine deps.
            warm = wp.tile([128, 2], f32, tag="warm")
            nc.gpsimd.memset(warm[:], 0.0)
            nc.scalar.activation(
                warm[:], warm[:], mybir.ActivationFunctionType.Copy
            )
            nc.vector.tensor_copy(warm[:], warm[:])

            w_sb = []
            for ki, (k0, ksz) in enumerate(_K_TILES):
                wt = wp.tile([128, G], bf16, tag=f"w{ki}")
                nc.gpsimd.dma_start(wt[:ksz, :], w01T[k0 : k0 + ksz, :])
                w_sb.append(wt)

            LOOK = 4
            DEFER = 1  # quads a store lags its compute: waits met on arrival
            # tks[q] = list of (tile, col_base) quarters covering the quad row
            tks = {}

            def emit_load(q):
                if q < NSPLIT:
                    quarters = []
                    for j in range(4):
                        tk = qt_pools[4 * q + j].tile(
                            [128, QCOL], bf16, name="tokq4", tag="tokq4"
                        )
                        nc.gpsimd.dma_start(
                            tk[:KP, :], tokq[q, :, j * QCOL : (j + 1) * QCOL]
                        )
                        quarters.append((tk, j * QCOL))
                    tks[q] = quarters
                else:
                    tk = tok_pools[(q - NSPLIT) % NTOK].tile(
                        [128, ROWE], bf16, name="tok", tag="tok"
                    )
                    nc.gpsimd.dma_start(tk[:KP, :], tokq[q, :, :ROWE])
                    tks[q] = [(tk, 0)]

            def rhs_slice(quarters, c0, csz):
                for tk, base in reversed(quarters):
                    if c0 >= base:
                        assert c0 + csz <= base + (
                            QCOL if len(quarters) > 1 else ROWE
                        )
                        return tk, c0 - base
                raise AssertionError

            pending_stores = []

            def flush_stores(keep=0):
                while len(pending_stores) > keep:
                    dst, src = pending_stores.pop(0)
                    nc.gpsimd.dma_start(dst, src)

            emit_load(0)
            loaded = 1

            cp = 0
            for q in range(NQ):
                for _ in range(2):
                    if loaded < NQ and loaded <= q + LOOK:
                        emit_load(loaded)
                        loaded += 1
                flush_stores(keep=DEFER)
                quarters = tks.pop(q)
                o = obp.tile([128, OCOL], bf16, name="ob", tag="ob")
                for h in range(2):
                    for mi in range(3):
                        pss = [
                            psp.tile([128, 896], f32, name="ps", tag="ps")
                            for _ in range(2)
                        ]
                        for ki, (k0, ksz) in enumerate(_K_TILES):
                            for bi in range(2):
                                c0 = (ki * QB + 2 * h + bi) * D
                                for n0, nsz in _N_TILES:
                                    tk, off = rhs_slice(quarters, c0 + n0, nsz)
                                    nc.tensor.matmul(
                                        pss[bi][:MP, n0 : n0 + nsz],
                                        w_sb[ki][:ksz, mi * MP : (mi + 1) * MP],
                                        tk[:ksz, off : off + nsz],
                                        start=(ki == 0),
                                        stop=(ki == 1),
                                    )
                        for bi in range(2):
                            b = 2 * h + bi
                            dst = o[:MP, (b * 3 + mi) * D : (b * 3 + mi + 1) * D]
                            if cp % 2 == 0:
                                nc.vector.tensor_copy(dst, pss[bi][:MP, :])
                            else:
                                nc.scalar.activation(
                                    dst,
                                    pss[bi][:MP, :],
                                    mybir.ActivationFunctionType.Copy,
                                )
                            cp += 1
                pending_stores.append((out[q, :MP, :], o[:MP, :]))
            flush_stores()

    nc.compile()
    _CACHE["nc"] = nc
    return nc


def _host_prep(tokens_full, large_mask, mid_mask):
    """Cast to bf16, pack tokens for quad loads, build weight matrix."""
    import ml_dtypes

    bf16 = ml_dtypes.bfloat16
    bsz = tokens_full.shape[0]
    tok_bf = np.asarray(tokens_full, np.float32).astype(bf16)

    # tokq[q, p, ki, b, d] = tok(4q+b, ki*113+p, d); k-chunk 1 row 112 is the
    # last valid row (225 = 113 + 112), partition 112 of chunk 1 zero-padded.
    t4 = tok_bf.reshape(bsz // QB, QB, T, D)
    tokq = np.zeros((bsz // QB, KP, ROWE + ROWPAD), bf16)
    tq = tokq[:, :, :ROWE].reshape(bsz // QB, KP, 2, QB, D)
    tq[:, :, 0] = t4[:, :, 0:KP].transpose(0, 2, 1, 3)
    tq[:, :112, 1] = t4[:, :, KP:T].transpose(0, 2, 1, 3)

    W = np.zeros((G, T), np.float64)
    rows = np.arange(GL)
    for l in range(large_mask.shape[0]):
        np.add.at(W, (rows, large_mask[l]), 1.0 / LL)
    rows = GL + np.arange(GM)
    for l in range(mid_mask.shape[0]):
        np.add.at(W, (rows, mid_mask[l]), 1.0 / LM)
    W[G - 1, :] = 1.0 / T

    # Permute groups so m-tile mi, partition p <-> group 3p+mi.
    perm = np.concatenate([np.arange(mi, G, 3) for mi in range(3)])
    w01T = np.ascontiguousarray(W[perm].T).astype(bf16)  # [T, G]
    return tokq, w01T


def _in_maps(tokq, w01T, n_cores=N_CORES):
    qp = tokq.shape[0] // n_cores
    return [
        {
            "tokq": np.ascontiguousarray(tokq[c * qp : (c + 1) * qp]),
            "w01T": w01T,
        }
        for c in range(n_cores)
    ]


def _unpack_out(res_out):
    """[NQ, 122+OPAD, QB*3*D] bf16 device layout -> [BP, G, D] f32."""
    arr = np.asarray(res_out).reshape(NQ, MP + OPAD, QB, 3, D)[:, :MP]
    return (
        arr.transpose(0, 2, 1, 3, 4)
        .reshape(BP, G, D)
        .astype(np.float32)
    )


def kernel(**inputs):
    from concourse import bass_utils

    tokens_full = np.ascontiguousarray(np.asarray(inputs["patch_tokens"], np.float32))
    large = np.asarray(inputs["large_mask"]).astype(np.int64)
    mid = np.asarray(inputs["mid_mask"]).astype(np.int64)
    tokq, w01T = _host_prep(tokens_full, large, mid)

    nc = _get_nc()
    res = bass_utils.run_bass_kernel_spmd(
        nc, _in_maps(tokq, w01T), core_ids=list(range(N_CORES))
    )
    return np.concatenate(
        [_unpack_out(res.results[c]["out"]) for c in range(N_CORES)], axis=0
    )

